# revision 8
# baseline (speedup 1.0000x reference)
"""Trainium2 Bass kernel for nn_Loss_10952166604854 (CenterNet-style loss).

Strategy (data-parallel over batch B=16 across 8 cores, 2 batches/core):

The dominant cost is the weighted-Hausdorff term: for every (s, b) pair a
(HW=16384 grid x K point) field of t^-9 where t = maxd + eps + p_g*(d_gk - maxd).

Device mapping per core (grid on partitions, compacted points on free axis):
  - K-compaction: masked-out points are removed on the host (they are excluded
    from the min by masking and from term2 by m=0), K -> KC ~ 70-80. Pad slots
    are "poisoned" with +BIG inside d^2 so they never win the min and add ~1e-27
    to the generalized-mean sums (discarded on host anyway).
  - d2[m, k] for grid column u: computed on PE as POLY^T @ RHS_b where
    POLY = [m^2; m; 1] (3 x 128, stationary) and
    RHS_b[:, u*KC+k] = [1; -2*y_k; y_k^2 + (u-x_k)^2 (+BIG)] (host-built).
  - d = sqrt(d2) on ACT (PSUM->SBUF), dmin2 = reduce_min over k on DVE (PSUM).
  - t = p_col * d + s2_col via per-u tensor_scalar (DVE/GPSIMD alternating),
    where s2 = maxd+eps - maxd*p (per-partition columns of the sigmoid image).
  - t^-9 = Exp(-9 * Ln(t)) as two whole-slab ACT passes (in-place).
  - sum_g t^-9 per k: PE ones-matmul accumulated in PSUM over grid chunks.
  - term1 pieces: sum_g p (accum_out of the clip op) and sum_g p*dmin
    (tensor_tensor_reduce) -> per-partition partials, finished on host.

The three scalar losses are assembled on the host from tiny per-core outputs;
the bounded-IoU term (gather + ~16K flops) is computed on the host directly.
"""

import os
import sys
from contextlib import ExitStack

import numpy as np

sys.path.insert(0, "/opt/trn_rl_repo")

import concourse.bacc as bacc
import concourse.bass as bass
import concourse.tile as tile
from concourse import mybir
from concourse.bass_utils import run_bass_kernel_spmd

F32 = mybir.dt.float32
AF = mybir.ActivationFunctionType
ALU = mybir.AluOpType

S, B, K, H, W = 2, 16, 128, 128, 128
NCORES = 8
BPC = B // NCORES          # batches per core
UNITS = S * BPC            # (s, b) pairs per core
HWX = H * W
EPS = 1e-6
P_GEN = -9.0
BETA = 0.2
HM_WEIGHT = 1.0
WH_WEIGHT = 0.1
MAXD = float(np.sqrt(np.float32(H * H + W * W)))   # 181.01933...
C2 = MAXD + EPS
BIG = 1.0e6
CLIP_LO = 1e-4
CLIP_HI = 1.0 - 1e-4

_module_cache = {}
STAGE = int(os.environ.get('KSTAGE', '5'))


def _build_module(KC):
    """Build + compile the per-core Bass program for point count KC."""
    UPC = max(1, 512 // KC)            # grid columns per PE/psum chunk
    NF = UPC * KC                      # full chunk free size (<= 512)
    chunks = []
    u0 = 0
    while u0 < W:
        cnt = min(UPC, W - u0)
        chunks.append((u0, cnt))
        u0 += cnt
    L = W * KC                         # free length of one (b) slab

    nc = bacc.Bacc("TRN2", target_bir_lowering=False, debug=False,
                   num_devices=NCORES)

    hm_in = nc.dram_tensor("hm_in", (UNITS, H, W), F32, kind="ExternalInput").ap()
    rhs_in = nc.dram_tensor("rhs_in", (BPC, 3, L), F32, kind="ExternalInput").ap()
    poly_in = nc.dram_tensor("poly_in", (3, H), F32, kind="ExternalInput").ap()
    ones_in = nc.dram_tensor("ones_in", (H, 1), F32, kind="ExternalInput").ap()
    out_sums = nc.dram_tensor("out_sums", (1, UNITS * NF), F32, kind="ExternalOutput").ap()
    out_t1 = nc.dram_tensor("out_t1", (H, 2 * UNITS), F32, kind="ExternalOutput").ap()

    with tile.TileContext(nc) as tc, ExitStack() as ctx:
        consts = ctx.enter_context(tc.tile_pool(name="consts", bufs=1))
        imgs = ctx.enter_context(tc.tile_pool(name="imgs", bufs=1))
        rhsp = ctx.enter_context(tc.tile_pool(name="rhsp", bufs=3))
        dpool = ctx.enter_context(tc.tile_pool(name="dpool", bufs=2))
        tpool = ctx.enter_context(tc.tile_pool(name="tpool", bufs=2))
        scrp = ctx.enter_context(tc.tile_pool(name="scrp", bufs=2))
        psA = ctx.enter_context(tc.tile_pool(name="psA", bufs=3, space="PSUM"))
        psB = ctx.enter_context(tc.tile_pool(name="psB", bufs=2, space="PSUM"))

        poly_sb = consts.tile([3, H], F32, tag="poly")
        nc.sync.dma_start(out=poly_sb, in_=poly_in)
        ones_sb = consts.tile([H, 1], F32, tag="ones")
        nc.sync.dma_start(out=ones_sb, in_=ones_in)

        p_all = imgs.tile([H, UNITS * W], F32, tag="p_all")
        s2_all = imgs.tile([H, UNITS * W], F32, tag="s2_all")
        dmin2 = imgs.tile([H, BPC * W], F32, tag="dmin2")
        t1_sb = imgs.tile([H, 2 * UNITS], F32, tag="t1_sb")
        sums_sb = imgs.tile([1, UNITS * NF], F32, tag="sums_sb")

        # ---- P-prep: sigmoid + clip (+n_est accum) + s2 image, per unit ----
        for i in range(UNITS):
            hm_t = scrp.tile([H, W], F32, tag="hm")
            nc.sync.dma_start(out=hm_t, in_=hm_in[i])
            ps = scrp.tile([H, W], F32, tag="ps")
            nc.scalar.activation(out=ps, in_=hm_t, func=AF.Sigmoid)
            pimg = p_all[:, bass.ts(i, W)]
            nc.vector.tensor_scalar(
                out=pimg, in0=ps, scalar1=CLIP_LO, scalar2=CLIP_HI,
                op0=ALU.max, op1=ALU.min)
            nc.vector.tensor_reduce(
                out=t1_sb[:, 2 * i + 1:2 * i + 2], in_=pimg,
                axis=mybir.AxisListType.X, op=ALU.add)
            nc.vector.tensor_scalar(
                out=s2_all[:, bass.ts(i, W)], in0=pimg,
                scalar1=-MAXD, scalar2=C2, op0=ALU.mult, op1=ALU.add)

        # ---- Phase A per b: d2 matmuls -> sqrt -> D slab; dmin2 ----
        d_slabs = []
        for b in range(BPC):
            D = dpool.tile([H, L], F32, tag="dslab")
            d_slabs.append(D)
            for (u0, cnt) in chunks:
                n = cnt * KC
                off = u0 * KC
                rchunk = rhsp.tile([3, NF], F32, tag="rhs")
                nc.sync.dma_start(out=rchunk[:, :n], in_=rhs_in[b, :, off:off + n])
                pa = psA.tile([H, NF], F32, tag="psA")
                nc.tensor.matmul(out=pa[:, :n], lhsT=poly_sb,
                                 rhs=rchunk[:, :n], start=True, stop=True)
                nc.scalar.activation(out=D[:, off:off + n], in_=pa[:, :n],
                                     func=AF.Sqrt)
                nc.vector.tensor_reduce(
                    out=dmin2[:, b * W + u0:b * W + u0 + cnt],
                    in_=pa[:, :n].rearrange("p (u k) -> p u k", k=KC),
                    axis=mybir.AxisListType.X, op=ALU.min)
            # dmin = sqrt(dmin2), in place
            nc.scalar.activation(out=dmin2[:, bass.ts(b, W)],
                                 in_=dmin2[:, bass.ts(b, W)], func=AF.Sqrt)

        # ---- Phase B per unit: t = p*d + s2; ln; exp(-9*); per-k sums ----
        for b in range(BPC) if STAGE >= 3 else []:
            D = d_slabs[b]
            for s in range(2):
                i = b * 2 + s
                if s == 0:
                    T = tpool.tile([H, L], F32, tag="tslab")
                else:
                    T = D  # reuse the d slab in place; D is dead afterwards
                pcolbase = i * W
                for u in range(W):
                    sl = slice(u * KC, (u + 1) * KC)
                    eng = nc.vector  # bisect: gpsimd TSS suspected in HW fault
                    eng.tensor_scalar(
                        out=T[:, sl], in0=D[:, sl],
                        scalar1=p_all[:, pcolbase + u:pcolbase + u + 1],
                        scalar2=s2_all[:, pcolbase + u:pcolbase + u + 1],
                        op0=ALU.mult, op1=ALU.add)
                if STAGE >= 4:
                    nc.scalar.activation(out=T, in_=T, func=AF.Ln)
                    nc.scalar.activation(out=T, in_=T, func=AF.Exp, scale=P_GEN)
                if STAGE >= 5:
                    pb = psB.tile([1, NF], F32, tag="psB")
                    for ci, (u0, cnt) in enumerate(chunks):
                        n = cnt * KC
                        nc.tensor.matmul(out=pb[:, :n], lhsT=ones_sb,
                                         rhs=T[:, u0 * KC:u0 * KC + n],
                                         start=(ci == 0), stop=(ci == len(chunks) - 1))
                    nc.vector.tensor_copy(out=sums_sb[:, i * NF:(i + 1) * NF], in_=pb)
                # term1 numerator partials: sum_g p * dmin
                scr = scrp.tile([H, W], F32, tag="ttr")
                nc.vector.tensor_mul(scr, p_all[:, bass.ts(i, W)],
                                     dmin2[:, bass.ts(b, W)])
                nc.vector.tensor_reduce(
                    out=t1_sb[:, 2 * i:2 * i + 1], in_=scr,
                    axis=mybir.AxisListType.X, op=ALU.add)

        nc.sync.dma_start(out=out_sums, in_=sums_sb)
        nc.sync.dma_start(out=out_t1, in_=t1_sb)

    nc.compile()
    meta = dict(KC=KC, NF=NF, UPC=UPC, chunks=chunks, L=L)
    return nc, meta


def _host_prep(hm, ctr, reg_mask):
    """Build per-core input maps. Returns (in_maps, per_b_info, KC)."""
    nb_all = reg_mask.sum(axis=1).astype(np.int64)          # (B,)
    KC = int(max(8, nb_all.max()))
    nc_mod, meta = _module_cache.get(KC, (None, None))
    if nc_mod is None:
        nc_mod, meta = _build_module(KC)
        _module_cache[KC] = (nc_mod, meta)
    L = meta["L"]

    m_idx = np.arange(H, dtype=np.float32)
    poly = np.stack([m_idx * m_idx, m_idx, np.ones(H, np.float32)]).astype(np.float32)
    ones = np.ones((H, 1), np.float32)

    in_maps = []
    info = []
    for c in range(NCORES):
        hmaps = np.empty((UNITS, H, W), np.float32)
        rhs = np.empty((BPC, 3, L), np.float32)
        core_info = []
        for bl in range(BPC):
            bg = c * BPC + bl
            ks = np.nonzero(reg_mask[bg])[0]
            nb = len(ks)
            ys = ctr[bg, ks, 1].astype(np.float32)
            xs = ctr[bg, ks, 0].astype(np.float32)
            ysp = np.zeros(KC, np.float32); ysp[:nb] = ys
            xsp = np.zeros(KC, np.float32); xsp[:nb] = xs
            pad = np.zeros(KC, np.float32); pad[nb:] = BIG
            u = np.arange(W, dtype=np.float32)[:, None]     # (W,1)
            r0 = np.ones((W, KC), np.float32)
            r1 = np.broadcast_to(-2.0 * ysp, (W, KC))
            r2 = ysp * ysp + (u - xsp) ** 2 + pad
            rhs[bl] = np.stack([r0, r1, r2]).reshape(3, L)
            for s in range(2):
                hmaps[bl * 2 + s] = hm[s, bg, 0]
            core_info.append((bg, nb))
        in_maps.append({"hm_in": hmaps, "rhs_in": rhs,
                        "poly_in": poly, "ones_in": ones})
        info.append(core_info)
    return nc_mod, meta, in_maps, info


def _host_finalize(results, meta, info):
    """Assemble hm_loss from per-core outputs."""
    KC, NF, UPC = meta["KC"], meta["NF"], meta["UPC"]
    hm_sum = 0.0
    for c in range(len(results)):
        sums = results[c]["out_sums"].reshape(UNITS, NF).astype(np.float64)
        t1 = results[c]["out_t1"].astype(np.float64)        # (H, 2*UNITS)
        for bl in range(BPC):
            bg, nb = info[c][bl]
            valid = 1.0 if nb > 0 else 0.0
            for s in range(2):
                i = bl * 2 + s
                row = sums[i]
                sk = row.reshape(UPC, KC).sum(axis=0)[:nb]  # per-point sums
                minn = (sk / HWX) ** (1.0 / P_GEN)
                term2 = minn.sum() / max(nb, 1.0)
                num1 = t1[:, 2 * i].sum()
                n_est = t1[:, 2 * i + 1].sum()
                term1 = num1 / (n_est + EPS)
                hm_sum += (term1 + term2) * valid
    return hm_sum / (B * S)


def _host_iou(wh_map, reg_map, reg_gt, wh_gt, ind, reg_mask):
    """Bounded-IoU loss, straight numpy port of the reference (tiny)."""
    def gather(feat):
        s, b, ch, h, w = feat.shape
        f = feat.reshape(s, b, ch, h * w).transpose(0, 1, 3, 2)   # (S,B,HW,C)
        idx = np.broadcast_to(ind[None, :, :, None], (s, b, ind.shape[1], ch))
        return np.take_along_axis(f, idx, axis=2)                 # (S,B,K,C)

    WH = gather(wh_map).astype(np.float32)
    REG = gather(reg_map).astype(np.float32)
    m = reg_mask.astype(np.float32)
    valid = (m.sum(-1) > 0).astype(np.float32)

    reg_g = reg_gt[None].astype(np.float32)
    wh_g = wh_gt[None].astype(np.float32)
    dx = reg_g[..., 0] - REG[..., 0]
    dy = reg_g[..., 1] - REG[..., 1]
    w_g, h_g = wh_g[..., 0], wh_g[..., 1]
    w_p, h_p = WH[..., 0], WH[..., 1]
    ldx = 1.0 - np.maximum((w_g - 2.0 * np.abs(dx)) / (w_g + 2.0 * np.abs(dx) + EPS), 0.0)
    ldy = 1.0 - np.maximum((h_g - 2.0 * np.abs(dy)) / (h_g + 2.0 * np.abs(dy) + EPS), 0.0)
    ldw = 1.0 - np.minimum(w_g / (w_p + EPS), w_p / (w_g + EPS))
    ldh = 1.0 - np.minimum(h_g / (h_p + EPS), h_p / (h_g + EPS))
    l = np.stack([ldx, ldy, ldw, ldh], -1)
    l = np.where(l < BETA, 0.5 * l * l / BETA, l - 0.5 * BETA)
    num = (l * m[None, :, :, None]).sum((-1, -2))
    den = np.maximum(m.sum(-1), 1.0)[None, :] * 4.0
    iou_terms = num / den
    return float((iou_terms * valid[None, :]).sum() / (B * S))


def _run(hm, wh_map, reg_map, reg_gt, wh_gt, ind, ctr, reg_mask, trace=False):
    nc_mod, meta, in_maps, info = _host_prep(hm, ctr, reg_mask)
    res = run_bass_kernel_spmd(nc_mod, in_maps, core_ids=list(range(NCORES)),
                               trace=trace)
    hm_loss = _host_finalize(res.results, meta, info)
    iou_loss = _host_iou(wh_map, reg_map, reg_gt, wh_gt, ind, reg_mask)
    loss = HM_WEIGHT * hm_loss + WH_WEIGHT * iou_loss
    out = np.array([loss, hm_loss, iou_loss], dtype=np.float32)
    return out, res


def kernel(hm, wh_map, reg_map, reg_gt, wh_gt, ind, ctr, reg_mask):
    out, _ = _run(hm, wh_map, reg_map, reg_gt, wh_gt, ind, ctr, reg_mask)
    return out
